# revision 1
# baseline (speedup 1.0000x reference)
"""Trainium2 Bass kernel for nn_CorrelationMapLayer.

reference semantics:
    d1 = bilinear_down28(feature1)            # [B, C, 28, 28]
    d2 = bilinear_down28(feature2)            # [B, C, 28, 28]
    f2_sel[b,c,k] = d2[b, c, y_k, x_k]        # knn gather (y=knn[:,1], x=knn[:,0])
    corr = relu(einsum('bck,bchw->bkhw', f2_sel, d1))
    out  = corr / sum_{h,w} exp(corr) * 10

Kernel structure (v2):
  * f2 branch: w-axis 2-tap downsample on DVE (premultiply + pair add, bf16),
    then PE transposes of the [C, 56*28] intermediate; the h-axis taps fold
    into a WEIGHTED selection matrix (2 nonzeros per knn column), so the
    h-downsample ops disappear entirely.
  * f1 branch: premultiply by the full 2D separable weight map (bf16), then
    the correlation matmul ACCUMULATES the four bilinear taps directly in
    PSUM via strided rhs views -> the [K, 56, 56] -> [K, 28, 28] output
    downsample costs zero vector work.
  * relu runs on ACT straight out of PSUM; exp+accumulate/reciprocal/scale
    as before.
  * Data parallel over batch: 4 batches per core x 8 cores.
"""

import os
import sys

import numpy as np

for _p in (
    "/root/.axon_site",
    "/root/.axon_site/_ro/trn_rl_repo",
    "/root/.axon_site/_ro/pypackages",
    "/opt/trn_rl_repo",
):
    if os.path.isdir(_p) and _p not in sys.path:
        sys.path.append(_p)

import concourse.bacc as bacc
import concourse.mybir as mybir
import concourse.tile as tile
from concourse import bass_utils

F32 = mybir.dt.float32
F32R = mybir.dt.float32r
BF16 = mybir.dt.bfloat16
AF = mybir.ActivationFunctionType

B, C, H, W, K = 32, 512, 56, 56, 100
NCORES = 8
BL = B // NCORES  # batches per core
S = 28
HW = H * W  # 3136
HW2 = H * S  # 1568 (w-downsampled width)
HW28 = S * S  # 784
NCB = C // 128  # 4 channel blocks
NT2 = (HW2 + 127) // 128  # 13 transpose chunks over 1568 (12 x 128 + 32)
NJH = 2  # output row-halves (28 = 2 x 14); psum chunk = 14*28 = 392


def _bilinear_matrix(in_size: int, out_size: int) -> np.ndarray:
    scale = np.float32((in_size - 1) / (out_size - 1)) if out_size > 1 else np.float32(0)
    coords = np.arange(out_size, dtype=np.float32) * scale
    lo = np.floor(coords).astype(np.int32)
    hi = np.minimum(lo + 1, in_size - 1)
    frac = coords - lo.astype(np.float32)
    M = np.zeros((out_size, in_size), np.float32)
    np.add.at(M, (np.arange(out_size), lo), np.float32(1.0) - frac)
    np.add.at(M, (np.arange(out_size), hi), frac)
    return M


def _tap_weights() -> np.ndarray:
    """wvec[w]: weight applied to input index w, whose (unique) consumer is
    output index w//2. Verifies the 2-tap stride-2 structure exactly."""
    M = _bilinear_matrix(H, S)  # [28, 56]
    wvec = np.zeros(H, np.float32)
    for w in range(H):
        wvec[w] = M[w // 2, w]
    M2 = np.zeros_like(M)
    for ow in range(S):
        M2[ow, 2 * ow] = wvec[2 * ow]
        M2[ow, 2 * ow + 1] = wvec[2 * ow + 1]
    assert np.abs(M - M2).max() <= 1e-6, "bilinear 2-tap structure violated"
    return wvec


_WVEC = _tap_weights()
# WW[p, h*56+w] = wvec[w]  (w-axis weights, replicated over h and partitions)
WW_NP = np.ascontiguousarray(
    np.broadcast_to(np.tile(_WVEC, H)[None, :], (128, HW)), dtype=np.float32
)
# WF[p, h*56+w] = wvec[h]*wvec[w]  (full separable 2D weight map, for f1)
_WF_ROW = (np.repeat(_WVEC, W) * np.tile(_WVEC, H)).astype(np.float32)
WF_NP = np.ascontiguousarray(np.broadcast_to(_WF_ROW[None, :], (128, HW)))
BF16_NP = mybir.dt.np(BF16)
IDENT_NP = np.ascontiguousarray(np.eye(128, dtype=BF16_NP))


def _sel_matrix(knn_inds: np.ndarray) -> np.ndarray:
    """Weighted selection matrix over the w-downsampled layout, chunked for
    128-partition tiles: for knn k at (x, y), rows flat = (2y+t)*28 + x with
    weight wvec[2y+t] (the h-axis bilinear taps). bf16."""
    knn = np.asarray(knn_inds)
    G2 = np.zeros((128, NT2 * K), np.float32)
    for k in range(knn.shape[0]):
        x = int(knn[k, 0])
        y = int(knn[k, 1])
        for th in (0, 1):
            h = 2 * y + th
            flat = h * S + x
            t, p = divmod(flat, 128)
            G2[p, t * K + k] += float(_WVEC[h])
    return np.ascontiguousarray(G2.astype(BF16_NP))


def _make_in_maps(f1: np.ndarray, f2: np.ndarray, knn_inds: np.ndarray):
    g2 = _sel_matrix(knn_inds)
    in_maps = []
    for c in range(NCORES):
        in_maps.append(
            {
                "f1": np.ascontiguousarray(f1[c * BL : (c + 1) * BL]),
                "f2": np.ascontiguousarray(f2[c * BL : (c + 1) * BL]),
                "ww": WW_NP,
                "wf": WF_NP,
                "g2": g2,
                "ident": IDENT_NP,
            }
        )
    return in_maps


def _build(tc, out_ap, f1_ap, f2_ap, ww_ap, wf_ap, g2_ap, ident_ap, reps=1):
    nc = tc.nc
    MS = __import__("concourse.bass", fromlist=["MemorySpace"]).MemorySpace

    from contextlib import ExitStack

    with ExitStack() as ctx:
        const = ctx.enter_context(tc.tile_pool(name="const", bufs=1))
        f2p = ctx.enter_context(tc.tile_pool(name="f2p", bufs=2))
        yp = ctx.enter_context(tc.tile_pool(name="yp", bufs=2))
        xwp = ctx.enter_context(tc.tile_pool(name="xwp", bufs=5))
        xwTp = ctx.enter_context(tc.tile_pool(name="xwTp", bufs=NT2 + 1))
        d2selp = ctx.enter_context(tc.tile_pool(name="d2selp", bufs=5))
        f1p = ctx.enter_context(tc.tile_pool(name="f1p", bufs=2))
        f1bp = ctx.enter_context(tc.tile_pool(name="f1bp", bufs=5))
        c28p = ctx.enter_context(tc.tile_pool(name="c28p", bufs=2))
        smallp = ctx.enter_context(tc.tile_pool(name="smallp", bufs=2))
        tpp = ctx.enter_context(tc.tile_pool(name="tpp", bufs=2, space=MS.PSUM))
        selpp = ctx.enter_context(tc.tile_pool(name="selpp", bufs=2, space=MS.PSUM))
        corrpp = ctx.enter_context(tc.tile_pool(name="corrpp", bufs=2, space=MS.PSUM))
        ww = const.tile([128, HW], F32, tag="ww")
        wf = const.tile([128, HW], F32, tag="wf")
        g2 = const.tile([128, NT2 * K], BF16, tag="g2")
        ident = const.tile([128, 128], BF16, tag="ident")
        nc.sync.dma_start(ww[:], ww_ap)
        nc.sync.dma_start(wf[:], wf_ap)
        nc.sync.dma_start(g2[:], g2_ap)
        nc.sync.dma_start(ident[:], ident_ap)

        for rep in range(reps):
          for b in range(BL):
              # ---- f2: load, w-axis premultiply+pair-add (bf16) ----
              xw_tiles = []
              for i in range(NCB):
                  tf2 = f2p.tile([128, HW], F32, tag="tf2")
                  nc.sync.dma_start(
                      tf2[:],
                      f2_ap[b, i * 128 : (i + 1) * 128, :, :].rearrange(
                          "c h w -> c (h w)"
                      ),
                  )
                  y = yp.tile([128, HW], BF16, tag="y")
                  eng = nc.gpsimd if i == 0 else nc.vector
                  eng.tensor_mul(y[:], tf2[:], ww[:])
                  yv = y.rearrange("c (h w) -> c h w", h=H)
                  xw = xwp.tile([128, HW2], BF16, tag="xw")
                  xwv = xw.rearrange("c (h o) -> c h o", h=H)
                  nc.vector.tensor_add(xwv, yv[:, :, 0:W:2], yv[:, :, 1:W:2])
                  xw_tiles.append(xw)

              # ---- PE transpose xw -> xwT chunks [hw2, c] (bf16) ----
              xwT_tiles = [
                  xwTp.tile([128, C], BF16, tag="xwT", name=f"xwT_{b}_{t}")
                  for t in range(NT2)
              ]
              for i in range(NCB):
                  for t in range(NT2):
                      wc = min(128, HW2 - 128 * t)
                      tp = tpp.tile([128, 128], BF16, tag="tp")
                      nc.tensor.transpose(
                          tp[0:wc, 0:128],
                          xw_tiles[i][:, t * 128 : t * 128 + wc],
                          ident[:],
                      )
                      if t % 2 == 0:
                          nc.scalar.copy(
                              xwT_tiles[t][0:wc, i * 128 : (i + 1) * 128],
                              tp[0:wc, 0:128],
                          )
                      else:
                          nc.vector.tensor_copy(
                              xwT_tiles[t][0:wc, i * 128 : (i + 1) * 128],
                              tp[0:wc, 0:128],
                          )

              # ---- weighted selection matmul (h-taps folded into g2) ----
              d2sel_tiles = []
              for i in range(NCB):
                  ps = selpp.tile([128, K], F32, tag="selps")
                  for t in range(NT2):
                      kk = min(128, HW2 - 128 * t)
                      nc.tensor.matmul(
                          ps[:],
                          xwT_tiles[t][0:kk, i * 128 : (i + 1) * 128],
                          g2[0:kk, t * K : (t + 1) * K],
                          start=(t == 0),
                          stop=(t == NT2 - 1),
                      )
                  d2sel = d2selp.tile([128, K], BF16, tag="d2sel")
                  nc.scalar.copy(d2sel[:], ps[:])
                  d2sel_tiles.append(d2sel)

              # ---- f1 load + full 2D weight premultiply (bf16) ----
              f1w_tiles = []
              for i in range(NCB):
                  tf1 = f1p.tile([128, HW], F32, tag="tf1")
                  nc.sync.dma_start(
                      tf1[:],
                      f1_ap[b, i * 128 : (i + 1) * 128, :, :].rearrange(
                          "c h w -> c (h w)"
                      ),
                  )
                  f1w = f1bp.tile([128, HW], BF16, tag="f1w")
                  eng = nc.gpsimd if i == 0 else nc.vector
                  eng.tensor_mul(f1w[:], tf1[:], wf[:])
                  f1w_tiles.append(f1w)

              # ---- correlation matmul with fused 4-tap 2D downsample ----
              # corr28[k, o, p] = sum_c sum_{t,s} d2sel[c,k] * f1w[c, 2o+t, 2p+s]
              c28 = c28p.tile([K, HW28], F32, tag="c28")
              for j in range(NJH):
                  cps = corrpp.tile([K, (S // NJH) * S], F32, tag="cps")
                  n = 0
                  nlast = NCB * 4 - 1
                  for i in range(NCB):
                      f1v = f1w_tiles[i].rearrange(
                          "c (o t p s) -> c o t p s", o=S, t=2, p=S, s=2
                      )
                      for th in range(2):
                          for sw in range(2):
                              rhs = f1v[
                                  :,
                                  j * (S // NJH) : (j + 1) * (S // NJH),
                                  th,
                                  :,
                                  sw,
                              ]
                              nc.tensor.matmul(
                                  cps[:],
                                  d2sel_tiles[i][:],
                                  rhs,
                                  start=(n == 0),
                                  stop=(n == nlast),
                              )
                              n += 1
                  # relu straight out of PSUM
                  nc.scalar.activation(
                      c28[:, j * (S // NJH) * S : (j + 1) * (S // NJH) * S],
                      cps[:],
                      AF.Relu,
                  )
              # exp + accumulate, reciprocal, scale by 10/denom
              expb = c28p.tile([K, HW28], F32, tag="c28", name=f"expb_{b}")
              den = smallp.tile([K, 1], F32, tag="den")
              nc.scalar.activation(expb[:], c28[:], AF.Exp, accum_out=den[:])
              rec = smallp.tile([K, 1], F32, tag="rec")
              nc.vector.reciprocal(rec[:], den[:])
              rec10 = smallp.tile([K, 1], F32, tag="rec10")
              nc.vector.tensor_scalar_mul(rec10[:], rec[:], 10.0)
              ob = c28p.tile([K, HW28], F32, tag="c28", name=f"ob_{b}")
              nc.scalar.mul(ob[:], c28[:], rec10[:])
              nc.sync.dma_start(out_ap[b], ob[:])


_CACHE: dict = {}


def _get_nc(reps=1):
    key = f"nc_{reps}"
    if key in _CACHE:
        return _CACHE[key]
    nc = bacc.Bacc(
        "TRN2",
        target_bir_lowering=False,
        debug=False,
        enable_asserts=False,
        num_devices=NCORES,
    )
    f1 = nc.dram_tensor("f1", [BL, C, H, W], F32, kind="ExternalInput").ap()
    f2 = nc.dram_tensor("f2", [BL, C, H, W], F32, kind="ExternalInput").ap()
    ww = nc.dram_tensor("ww", [128, HW], F32, kind="ExternalInput").ap()
    wf = nc.dram_tensor("wf", [128, HW], F32, kind="ExternalInput").ap()
    g2 = nc.dram_tensor("g2", [128, NT2 * K], BF16, kind="ExternalInput").ap()
    ident = nc.dram_tensor("ident", [128, 128], BF16, kind="ExternalInput").ap()
    out = nc.dram_tensor("out", [BL, K, HW28], F32, kind="ExternalOutput").ap()
    with tile.TileContext(nc) as tc:
        _build(tc, out, f1, f2, ww, wf, g2, ident, reps=reps)
    nc.compile()
    _CACHE[key] = nc
    return nc


def kernel(feature1, feature2, knn_inds):
    f1 = np.ascontiguousarray(np.asarray(feature1, dtype=np.float32))
    f2 = np.ascontiguousarray(np.asarray(feature2, dtype=np.float32))
    nc = _get_nc()
    in_maps = _make_in_maps(f1, f2, knn_inds)
    res = bass_utils.run_bass_kernel_spmd(nc, in_maps, core_ids=list(range(NCORES)))
    _CACHE["last_results"] = res
    out = np.concatenate([r["out"] for r in res.results], axis=0)
    return out.reshape(B, K, S, S)



# revision 5
# speedup vs baseline: 1.4729x; 1.4729x over previous
"""Trainium2 Bass kernel for nn_CorrelationMapLayer.

reference semantics:
    d1 = bilinear_down28(feature1)            # [B, C, 28, 28]
    d2 = bilinear_down28(feature2)            # [B, C, 28, 28]
    f2_sel[b,c,k] = d2[b, c, y_k, x_k]        # knn gather (y=knn[:,1], x=knn[:,0])
    corr = relu(einsum('bck,bchw->bkhw', f2_sel, d1))
    out  = corr / sum_{h,w} exp(corr) * 10

Kernel structure (v3):
  * inputs are cast to bf16 on the host -> HBM traffic halves (DMA is the
    roofline: ~26 MB/core).
  * f2 branch: full 2D separable premultiply (DVE 2x), h-pair add (packed
    last dim -> DVE 2x), w-pair add (strided -> gpsimd, which is otherwise
    idle), 7 flat PE transposes of the contiguous d2 [c, 784], one-hot
    selection matmul -> d2sel [c, K].
  * f1 branch: NO elementwise work. Raw bf16 tiles feed the correlation
    matmul in the original 56x56 space; the bilinear downsample is applied
    AFTER the matmul on corr56 [K=100, 3136] (K < C so this is ~2.3x
    cheaper): ACT copies psum->bf16, premultiply by the same separable
    weight map (DVE 2x), h-pair add (2x), strided w-pair add -> craw,
    one relu, exp+accumulate, reciprocal, scale.
  * Data parallel over batch: 4 batches per core x 8 cores.
"""

import os
import sys

import numpy as np

for _p in (
    "/root/.axon_site",
    "/root/.axon_site/_ro/trn_rl_repo",
    "/root/.axon_site/_ro/pypackages",
    "/opt/trn_rl_repo",
):
    if os.path.isdir(_p) and _p not in sys.path:
        sys.path.append(_p)

import concourse.bacc as bacc
import concourse.mybir as mybir
import concourse.tile as tile
from concourse import bass_utils

F32 = mybir.dt.float32
BF16 = mybir.dt.bfloat16
AF = mybir.ActivationFunctionType

B, C, H, W, K = 32, 512, 56, 56, 100
NCORES = 8
BL = B // NCORES  # batches per core
S = 28
HW = H * W  # 3136
HW28 = S * S  # 784
NCB = C // 128  # 4 channel blocks
NCHUNK = 7  # transpose chunks over 784 columns: 6 x 128 + 16
# corr psum tiles cover h-row groups: 3 tiles of 16 rows + 1 tile of 8 rows
CORR_TILES = [(0, 16), (16, 16), (32, 16), (48, 8)]

BF16_NP = mybir.dt.np(BF16)


def _bilinear_matrix(in_size: int, out_size: int) -> np.ndarray:
    scale = np.float32((in_size - 1) / (out_size - 1)) if out_size > 1 else np.float32(0)
    coords = np.arange(out_size, dtype=np.float32) * scale
    lo = np.floor(coords).astype(np.int32)
    hi = np.minimum(lo + 1, in_size - 1)
    frac = coords - lo.astype(np.float32)
    M = np.zeros((out_size, in_size), np.float32)
    np.add.at(M, (np.arange(out_size), lo), np.float32(1.0) - frac)
    np.add.at(M, (np.arange(out_size), hi), frac)
    return M


def _tap_weights() -> np.ndarray:
    """wvec[w]: weight applied to input index w, whose (unique) consumer is
    output index w//2. Verifies the 2-tap stride-2 structure exactly."""
    M = _bilinear_matrix(H, S)  # [28, 56]
    wvec = np.zeros(H, np.float32)
    for w in range(H):
        wvec[w] = M[w // 2, w]
    M2 = np.zeros_like(M)
    for ow in range(S):
        M2[ow, 2 * ow] = wvec[2 * ow]
        M2[ow, 2 * ow + 1] = wvec[2 * ow + 1]
    assert np.abs(M - M2).max() <= 1e-6, "bilinear 2-tap structure violated"
    return wvec


_WVEC = _tap_weights()
# WF[p, h*56+w] = wvec[h]*wvec[w]  (full separable 2D weight map)
_WF_ROW = (np.repeat(_WVEC, W) * np.tile(_WVEC, H)).astype(np.float32)
WF_NP = np.ascontiguousarray(
    np.broadcast_to(_WF_ROW[None, :], (128, HW)), dtype=BF16_NP
)
IDENT_NP = np.ascontiguousarray(np.eye(128, dtype=BF16_NP))


def _sel_matrix(knn_inds: np.ndarray) -> np.ndarray:
    """One-hot selection over the fully downsampled flat 28x28 map, chunked
    flat: for knn k at (x, y), flat = y*28 + x lives in chunk flat//128 at
    partition flat%128; column chunk*K + k."""
    knn = np.asarray(knn_inds)
    G2 = np.zeros((128, NCHUNK * K), np.float32)
    for k in range(knn.shape[0]):
        x = int(knn[k, 0])
        y = int(knn[k, 1])
        flat = y * S + x
        r, p = divmod(flat, 128)
        G2[p, r * K + k] += 1.0
    return np.ascontiguousarray(G2.astype(BF16_NP))


def _make_in_maps(f1: np.ndarray, f2: np.ndarray, knn_inds: np.ndarray):
    g2 = _sel_matrix(knn_inds)
    in_maps = []
    for c in range(NCORES):
        in_maps.append(
            {
                "f1": np.ascontiguousarray(f1[c * BL : (c + 1) * BL]),
                "f2": np.ascontiguousarray(f2[c * BL : (c + 1) * BL]),
                "wf": WF_NP,
                "g2": g2,
                "ident": IDENT_NP,
            }
        )
    return in_maps


def _build(tc, out_ap, f1_ap, f2_ap, wf_ap, g2_ap, ident_ap):
    nc = tc.nc
    MS = __import__("concourse.bass", fromlist=["MemorySpace"]).MemorySpace

    from contextlib import ExitStack

    with ExitStack() as ctx:
        const = ctx.enter_context(tc.tile_pool(name="const", bufs=1))
        tf2p = ctx.enter_context(tc.tile_pool(name="tf2p", bufs=2))
        yp = ctx.enter_context(tc.tile_pool(name="yp", bufs=2))
        zp = ctx.enter_context(tc.tile_pool(name="zp", bufs=2))
        d2p = ctx.enter_context(tc.tile_pool(name="d2p", bufs=2))
        xwTp = ctx.enter_context(tc.tile_pool(name="xwTp", bufs=2))
        d2selp = ctx.enter_context(tc.tile_pool(name="d2selp", bufs=5))
        tf1p = ctx.enter_context(tc.tile_pool(name="tf1p", bufs=8))
        cbp = ctx.enter_context(tc.tile_pool(name="cbp", bufs=2))
        up = ctx.enter_context(tc.tile_pool(name="up", bufs=2))
        vp = ctx.enter_context(tc.tile_pool(name="vp", bufs=2))
        crawp = ctx.enter_context(tc.tile_pool(name="crawp", bufs=2))
        c28p = ctx.enter_context(tc.tile_pool(name="c28p", bufs=2))
        expbp = ctx.enter_context(tc.tile_pool(name="expbp", bufs=2))
        obp = ctx.enter_context(tc.tile_pool(name="obp", bufs=2))
        smallp = ctx.enter_context(tc.tile_pool(name="smallp", bufs=6))
        tpp = ctx.enter_context(tc.tile_pool(name="tpp", bufs=2, space=MS.PSUM))
        selpp = ctx.enter_context(tc.tile_pool(name="selpp", bufs=1, space=MS.PSUM))
        cpsp = ctx.enter_context(tc.tile_pool(name="cpsp", bufs=2, space=MS.PSUM))
        cps1p = ctx.enter_context(tc.tile_pool(name="cps1p", bufs=1, space=MS.PSUM))

        wf = const.tile([128, HW], BF16, tag="wf")
        g2 = const.tile([128, NCHUNK * K], BF16, tag="g2")
        ident = const.tile([128, 128], BF16, tag="ident")
        nc.sync.dma_start(wf[:], wf_ap)
        nc.sync.dma_start(g2[:], g2_ap)
        nc.sync.dma_start(ident[:], ident_ap)

        for b in range(BL):
            # ---- f2 branch: full downsample -> transpose -> select ----
            d2sel_tiles = []
            for i in range(NCB):
                tf2 = tf2p.tile([128, HW], BF16, tag="tf2")
                nc.sync.dma_start(
                    tf2[:],
                    f2_ap[b, i * 128 : (i + 1) * 128, :, :].rearrange(
                        "c h w -> c (h w)"
                    ),
                )
                y = yp.tile([128, HW], BF16, tag="y")
                nc.vector.tensor_mul(y[:], tf2[:], wf[:])
                yv = y.rearrange("c (h w) -> c h w", h=H)
                z = zp.tile([128, S * W], BF16, tag="z")
                zv = z.rearrange("c (h w) -> c h w", h=S)
                nc.vector.tensor_add(zv, yv[:, 0:H:2, :], yv[:, 1:H:2, :])
                d2 = d2p.tile([128, HW28], BF16, tag="d2")
                d2v = d2.rearrange("c (h w) -> c h w", h=S)
                nc.gpsimd.tensor_add(d2v, zv[:, :, 0:W:2], zv[:, :, 1:W:2])
                # 7 flat PE transposes of d2 (6 x 128 + 16 columns)
                tp = tpp.tile([128, NCHUNK * 128], BF16, tag="tp")
                for r in range(NCHUNK):
                    wc = min(128, HW28 - r * 128)
                    nc.tensor.transpose(
                        tp[0:wc, r * 128 : r * 128 + 128],
                        d2[:, r * 128 : r * 128 + wc],
                        ident[:],
                    )
                xwT = xwTp.tile([128, NCHUNK * 128], BF16, tag="xwT")
                nc.scalar.copy(xwT[:, 0 : 6 * 128], tp[:, 0 : 6 * 128])
                nc.scalar.copy(
                    xwT[0:16, 6 * 128 : 7 * 128], tp[0:16, 6 * 128 : 7 * 128]
                )
                ps = selpp.tile([128, K], F32, tag="selps")
                for r in range(NCHUNK):
                    wc = min(128, HW28 - r * 128)
                    nc.tensor.matmul(
                        ps[:],
                        xwT[0:wc, r * 128 : r * 128 + 128],
                        g2[0:wc, r * K : (r + 1) * K],
                        start=(r == 0),
                        stop=(r == NCHUNK - 1),
                    )
                dsel = d2selp.tile([128, K], BF16, tag="d2sel")
                nc.scalar.copy(dsel[:], ps[:])
                d2sel_tiles.append(dsel)

            # ---- f1 loads (raw bf16, no elementwise work) ----
            tf1_tiles = []
            for i in range(NCB):
                tf1 = tf1p.tile([128, HW], BF16, tag="tf1")
                nc.sync.dma_start(
                    tf1[:],
                    f1_ap[b, i * 128 : (i + 1) * 128, :, :].rearrange(
                        "c h w -> c (h w)"
                    ),
                )
                tf1_tiles.append(tf1.rearrange("c (h w) -> c h w", h=H))

            # ---- correlation in 56x56 space + post-matmul downsample ----
            craw = crawp.tile([K, HW28], F32, tag="craw")
            cr3 = craw.rearrange("k (h w) -> k h w", h=S)
            for h0, nh in CORR_TILES:
                if nh == 16:
                    cps = cpsp.tile([K, 1024], F32, tag="cps")
                else:
                    cps = cps1p.tile([K, 8 * W], F32, tag="cps1")
                for half in range(nh // 8):
                    for i in range(NCB):
                        nc.tensor.matmul(
                            cps[:, half * 512 : half * 512 + 8 * W],
                            d2sel_tiles[i][:],
                            tf1_tiles[i][:, h0 + half * 8 : h0 + half * 8 + 8, :],
                            start=(i == 0),
                            stop=(i == NCB - 1),
                        )
                cb = cbp.tile([K, 16 * W], BF16, tag="cb")
                if nh == 16:
                    cpsv = cps.rearrange("k (t x) -> k t x", t=2)
                    cbv = cb.rearrange("k (t x) -> k t x", t=2)
                    nc.scalar.copy(cbv[:, :, 0 : 8 * W], cpsv[:, :, 0 : 8 * W])
                else:
                    nc.scalar.copy(cb[:, 0 : nh * W], cps[:, 0 : nh * W])
                u = up.tile([K, 16 * W], BF16, tag="u")
                nc.vector.tensor_mul(
                    u[:, 0 : nh * W],
                    cb[:, 0 : nh * W],
                    wf[0:K, h0 * W : (h0 + nh) * W],
                )
                u3 = u.rearrange("k (h w) -> k h w", h=16)
                v = vp.tile([K, 8 * W], BF16, tag="v")
                v3 = v.rearrange("k (h w) -> k h w", h=8)
                nc.vector.tensor_add(
                    v3[:, 0 : nh // 2, :], u3[:, 0:nh:2, :], u3[:, 1:nh:2, :]
                )
                nc.vector.tensor_add(
                    cr3[:, h0 // 2 : (h0 + nh) // 2, :],
                    v3[:, 0 : nh // 2, 0:W:2],
                    v3[:, 0 : nh // 2, 1:W:2],
                )

            # ---- relu, exp + accumulate, reciprocal, scale by 10/denom ----
            c28 = c28p.tile([K, HW28], F32, tag="c28")
            nc.scalar.activation(c28[:], craw[:], AF.Relu)
            expb = expbp.tile([K, HW28], BF16, tag="expb")
            den = smallp.tile([K, 1], F32, tag="den")
            nc.scalar.activation(expb[:], c28[:], AF.Exp, accum_out=den[:])
            rec = smallp.tile([K, 1], F32, tag="rec")
            nc.vector.reciprocal(rec[:], den[:])
            rec10 = smallp.tile([K, 1], F32, tag="rec10")
            nc.vector.tensor_scalar_mul(rec10[:], rec[:], 10.0)
            ob = obp.tile([K, HW28], F32, tag="ob")
            nc.scalar.mul(ob[:], c28[:], rec10[:])
            nc.sync.dma_start(out_ap[b], ob[:])


_CACHE: dict = {}


def _get_nc():
    if "nc" in _CACHE:
        return _CACHE["nc"]
    nc = bacc.Bacc(
        "TRN2",
        target_bir_lowering=False,
        debug=False,
        enable_asserts=False,
        num_devices=NCORES,
    )
    f1 = nc.dram_tensor("f1", [BL, C, H, W], BF16, kind="ExternalInput").ap()
    f2 = nc.dram_tensor("f2", [BL, C, H, W], BF16, kind="ExternalInput").ap()
    wf = nc.dram_tensor("wf", [128, HW], BF16, kind="ExternalInput").ap()
    g2 = nc.dram_tensor("g2", [128, NCHUNK * K], BF16, kind="ExternalInput").ap()
    ident = nc.dram_tensor("ident", [128, 128], BF16, kind="ExternalInput").ap()
    out = nc.dram_tensor("out", [BL, K, HW28], F32, kind="ExternalOutput").ap()
    with tile.TileContext(nc) as tc:
        _build(tc, out, f1, f2, wf, g2, ident)
    nc.compile()
    _CACHE["nc"] = nc
    return nc


def kernel(feature1, feature2, knn_inds):
    f1 = np.asarray(feature1, dtype=np.float32).astype(BF16_NP)
    f2 = np.asarray(feature2, dtype=np.float32).astype(BF16_NP)
    nc = _get_nc()
    in_maps = _make_in_maps(f1, f2, knn_inds)
    res = bass_utils.run_bass_kernel_spmd(nc, in_maps, core_ids=list(range(NCORES)))
    _CACHE["last_results"] = res
    out = np.concatenate([r["out"] for r in res.results], axis=0)
    return out.reshape(B, K, S, S)


# revision 12
# speedup vs baseline: 1.7431x; 1.1835x over previous
"""Trainium2 Bass kernel for nn_CorrelationMapLayer.

reference semantics:
    d1 = bilinear_down28(feature1)            # [B, C, 28, 28]
    d2 = bilinear_down28(feature2)            # [B, C, 28, 28]
    f2_sel[b,c,k] = d2[b, c, y_k, x_k]        # knn gather (y=knn[:,1], x=knn[:,0])
    corr = relu(einsum('bck,bchw->bkhw', f2_sel, d1))
    out  = corr / sum_{h,w} exp(corr) * 10

Kernel structure (v3):
  * inputs are cast to bf16 on the host -> HBM traffic halves (DMA is the
    roofline: ~26 MB/core).
  * f2 branch: full 2D separable premultiply (DVE 2x), h-pair add (packed
    last dim -> DVE 2x), w-pair add (strided -> gpsimd, which is otherwise
    idle), 7 flat PE transposes of the contiguous d2 [c, 784], one-hot
    selection matmul -> d2sel [c, K].
  * f1 branch: NO elementwise work. Raw bf16 tiles feed the correlation
    matmul in the original 56x56 space; the bilinear downsample is applied
    AFTER the matmul on corr56 [K=100, 3136] (K < C so this is ~2.3x
    cheaper): ACT copies psum->bf16, premultiply by the same separable
    weight map (DVE 2x), h-pair add (2x), strided w-pair add -> craw,
    one relu, exp+accumulate, reciprocal, scale.
  * Data parallel over batch: 4 batches per core x 8 cores.
"""

import os
import sys

import numpy as np

for _p in (
    "/root/.axon_site",
    "/root/.axon_site/_ro/trn_rl_repo",
    "/root/.axon_site/_ro/pypackages",
    "/opt/trn_rl_repo",
):
    if os.path.isdir(_p) and _p not in sys.path:
        sys.path.append(_p)

import concourse.bacc as bacc
import concourse.mybir as mybir
import concourse.tile as tile
from concourse import bass_utils

F32 = mybir.dt.float32
BF16 = mybir.dt.bfloat16
AF = mybir.ActivationFunctionType

B, C, H, W, K = 32, 512, 56, 56, 100
NCORES = 8
BL = B // NCORES  # batches per core
S = 28
HW = H * W  # 3136
HW28 = S * S  # 784
NCB = C // 128  # 4 channel blocks
NCHUNK = 7  # transpose chunks over 784 columns: 6 x 128 + 16
# corr psum tiles cover h-row groups: 3 tiles of 16 rows + 1 tile of 8 rows
CORR_TILES = [(0, 16), (16, 16), (32, 16), (48, 8)]

BF16_NP = mybir.dt.np(BF16)


def _bilinear_matrix(in_size: int, out_size: int) -> np.ndarray:
    scale = np.float32((in_size - 1) / (out_size - 1)) if out_size > 1 else np.float32(0)
    coords = np.arange(out_size, dtype=np.float32) * scale
    lo = np.floor(coords).astype(np.int32)
    hi = np.minimum(lo + 1, in_size - 1)
    frac = coords - lo.astype(np.float32)
    M = np.zeros((out_size, in_size), np.float32)
    np.add.at(M, (np.arange(out_size), lo), np.float32(1.0) - frac)
    np.add.at(M, (np.arange(out_size), hi), frac)
    return M


def _tap_weights() -> np.ndarray:
    """wvec[w]: weight applied to input index w, whose (unique) consumer is
    output index w//2. Verifies the 2-tap stride-2 structure exactly."""
    M = _bilinear_matrix(H, S)  # [28, 56]
    wvec = np.zeros(H, np.float32)
    for w in range(H):
        wvec[w] = M[w // 2, w]
    M2 = np.zeros_like(M)
    for ow in range(S):
        M2[ow, 2 * ow] = wvec[2 * ow]
        M2[ow, 2 * ow + 1] = wvec[2 * ow + 1]
    assert np.abs(M - M2).max() <= 1e-6, "bilinear 2-tap structure violated"
    return wvec


_WVEC = _tap_weights()
# WF[p, h*56+w] = wvec[h]*wvec[w]  (full separable 2D weight map)
_WF_ROW = (np.repeat(_WVEC, W) * np.tile(_WVEC, H)).astype(np.float32)
WF_NP = np.ascontiguousarray(
    np.broadcast_to(_WF_ROW[None, :], (128, HW)), dtype=BF16_NP
)
IDENT_NP = np.ascontiguousarray(np.eye(128, dtype=BF16_NP))


def _sel_matrix(knn_inds: np.ndarray) -> np.ndarray:
    """One-hot selection over the fully downsampled flat 28x28 map, chunked
    flat: for knn k at (x, y), flat = y*28 + x lives in chunk flat//128 at
    partition flat%128; column chunk*K + k."""
    knn = np.asarray(knn_inds)
    G2 = np.zeros((128, NCHUNK * K), np.float32)
    for k in range(knn.shape[0]):
        x = int(knn[k, 0])
        y = int(knn[k, 1])
        flat = y * S + x
        r, p = divmod(flat, 128)
        G2[p, r * K + k] += 1.0
    return np.ascontiguousarray(G2.astype(BF16_NP))


def _make_in_maps(f1: np.ndarray, f2: np.ndarray, knn_inds: np.ndarray):
    g2 = _sel_matrix(knn_inds)
    in_maps = []
    for c in range(NCORES):
        in_maps.append(
            {
                "f1": np.ascontiguousarray(f1[c * BL : (c + 1) * BL]),
                "f2": np.ascontiguousarray(f2[c * BL : (c + 1) * BL]),
                "wf": WF_NP,
                "g2": g2,
                "ident": IDENT_NP,
            }
        )
    return in_maps


def _build(tc, out_ap, f1_ap, f2_ap, wf_ap, g2_ap, ident_ap):
    nc = tc.nc
    MS = __import__("concourse.bass", fromlist=["MemorySpace"]).MemorySpace

    from contextlib import ExitStack

    with ExitStack() as ctx:
        const = ctx.enter_context(tc.tile_pool(name="const", bufs=1))
        tf2p = ctx.enter_context(tc.tile_pool(name="tf2p", bufs=2))
        yp = ctx.enter_context(tc.tile_pool(name="yp", bufs=2))
        zp = ctx.enter_context(tc.tile_pool(name="zp", bufs=2))
        d2p = ctx.enter_context(tc.tile_pool(name="d2p", bufs=2))
        xwTp = ctx.enter_context(tc.tile_pool(name="xwTp", bufs=2))
        d2selp = ctx.enter_context(tc.tile_pool(name="d2selp", bufs=8))
        tf1p = ctx.enter_context(tc.tile_pool(name="tf1p", bufs=8))
        cbp = ctx.enter_context(tc.tile_pool(name="cbp", bufs=2))
        up = ctx.enter_context(tc.tile_pool(name="up", bufs=2))
        vp = ctx.enter_context(tc.tile_pool(name="vp", bufs=2))
        crawp = ctx.enter_context(tc.tile_pool(name="crawp", bufs=2))
        c28p = ctx.enter_context(tc.tile_pool(name="c28p", bufs=2))
        expbp = ctx.enter_context(tc.tile_pool(name="expbp", bufs=2))
        obp = ctx.enter_context(tc.tile_pool(name="obp", bufs=2))
        smallp = ctx.enter_context(tc.tile_pool(name="smallp", bufs=6))
        tpp = ctx.enter_context(tc.tile_pool(name="tpp", bufs=2, space=MS.PSUM))
        selpp = ctx.enter_context(tc.tile_pool(name="selpp", bufs=2, space=MS.PSUM))
        cpsp = ctx.enter_context(tc.tile_pool(name="cpsp", bufs=3, space=MS.PSUM))

        wf = const.tile([128, HW], BF16, tag="wf")
        g2 = const.tile([128, NCHUNK * K], BF16, tag="g2")
        ident = const.tile([128, 128], BF16, tag="ident")
        nc.sync.dma_start(wf[:], wf_ap)
        nc.sync.dma_start(g2[:], g2_ap)
        nc.sync.dma_start(ident[:], ident_ap)

        for b in range(BL):
            # ---- f2 branch: full downsample -> transpose -> select ----
            d2sel_tiles = []
            for i in range(NCB):
                tf2 = tf2p.tile([128, HW], BF16, tag="tf2")
                nc.sync.dma_start(
                    tf2[:],
                    f2_ap[b, i * 128 : (i + 1) * 128, :, :].rearrange(
                        "c h w -> c (h w)"
                    ),
                )
                y = yp.tile([128, HW], BF16, tag="y")
                nc.vector.tensor_mul(y[:], tf2[:], wf[:])
                yv = y.rearrange("c (h w) -> c h w", h=H)
                z = zp.tile([128, S * W], BF16, tag="z")
                zv = z.rearrange("c (h w) -> c h w", h=S)
                nc.vector.tensor_add(zv, yv[:, 0:H:2, :], yv[:, 1:H:2, :])
                d2 = d2p.tile([128, HW28], BF16, tag="d2")
                d2v = d2.rearrange("c (h w) -> c h w", h=S)
                nc.gpsimd.tensor_add(d2v, zv[:, :, 0:W:2], zv[:, :, 1:W:2])
                # 7 flat PE transposes of d2 (6 x 128 + 16 columns)
                tp = tpp.tile([128, NCHUNK * 128], BF16, tag="tp")
                for r in range(NCHUNK):
                    wc = min(128, HW28 - r * 128)
                    nc.tensor.transpose(
                        tp[0:wc, r * 128 : r * 128 + 128],
                        d2[:, r * 128 : r * 128 + wc],
                        ident[:],
                    )
                xwT = xwTp.tile([128, NCHUNK * 128], BF16, tag="xwT")
                if i % 2 == 0:
                    nc.scalar.copy(xwT[:, 0 : 6 * 128], tp[:, 0 : 6 * 128])
                    nc.scalar.copy(
                        xwT[0:16, 6 * 128 : 7 * 128], tp[0:16, 6 * 128 : 7 * 128]
                    )
                else:
                    nc.vector.tensor_copy(xwT[:, 0 : 6 * 128], tp[:, 0 : 6 * 128])
                    nc.vector.tensor_copy(
                        xwT[0:16, 6 * 128 : 7 * 128], tp[0:16, 6 * 128 : 7 * 128]
                    )
                ps = selpp.tile([128, K], F32, tag="selps")
                for r in range(NCHUNK):
                    wc = min(128, HW28 - r * 128)
                    nc.tensor.matmul(
                        ps[:],
                        xwT[0:wc, r * 128 : r * 128 + 128],
                        g2[0:wc, r * K : (r + 1) * K],
                        start=(r == 0),
                        stop=(r == NCHUNK - 1),
                    )
                dsel = d2selp.tile([128, K], BF16, tag="d2sel")
                nc.scalar.copy(dsel[:], ps[:])
                d2sel_tiles.append(dsel)

            # ---- f1 loads (raw bf16, no elementwise work) ----
            tf1_tiles = []
            for i in range(NCB):
                tf1 = tf1p.tile([128, HW], BF16, tag="tf1")
                nc.sync.dma_start(
                    tf1[:],
                    f1_ap[b, i * 128 : (i + 1) * 128, :, :].rearrange(
                        "c h w -> c (h w)"
                    ),
                )
                tf1_tiles.append(tf1.rearrange("c (h w) -> c h w", h=H))

            # ---- correlation in 56x56 space + post-matmul downsample ----
            craw = crawp.tile([K, HW28], F32, tag="craw")
            cr3 = craw.rearrange("k (h w) -> k h w", h=S)
            for g in range(NCHUNK):  # 7 groups of 8 h-rows
                h0 = g * 8
                cps = cpsp.tile([K, 8 * W], F32, tag="cps")
                for i in range(NCB):
                    nc.tensor.matmul(
                        cps[:],
                        d2sel_tiles[i][:],
                        tf1_tiles[i][:, h0 : h0 + 8, :],
                        start=(i == 0),
                        stop=(i == NCB - 1),
                    )
                cb = cbp.tile([K, 8 * W], BF16, tag="cb")
                if g % 3 == 2:
                    nc.vector.tensor_copy(cb[:], cps[:])
                else:
                    nc.scalar.copy(cb[:], cps[:])
                u = up.tile([K, 8 * W], BF16, tag="u")
                nc.vector.tensor_mul(
                    u[:], cb[:], wf[0:K, h0 * W : (h0 + 8) * W]
                )
                u3 = u.rearrange("k (h w) -> k h w", h=8)
                v = vp.tile([K, 4 * W], BF16, tag="v")
                v3 = v.rearrange("k (h w) -> k h w", h=4)
                nc.vector.tensor_add(v3, u3[:, 0:8:2, :], u3[:, 1:8:2, :])
                nc.vector.tensor_add(
                    cr3[:, g * 4 : (g + 1) * 4, :],
                    v3[:, :, 0:W:2],
                    v3[:, :, 1:W:2],
                )

            # ---- relu, exp + accumulate, reciprocal, scale by 10/denom ----
            c28 = c28p.tile([K, HW28], F32, tag="c28")
            nc.scalar.activation(c28[:], craw[:], AF.Relu)
            expb = expbp.tile([K, HW28], BF16, tag="expb")
            den = smallp.tile([K, 1], F32, tag="den")
            nc.scalar.activation(expb[:], c28[:], AF.Exp, accum_out=den[:])
            rec = smallp.tile([K, 1], F32, tag="rec")
            nc.vector.reciprocal(rec[:], den[:])
            rec10 = smallp.tile([K, 1], F32, tag="rec10")
            nc.vector.tensor_scalar_mul(rec10[:], rec[:], 10.0)
            ob = obp.tile([K, HW28], F32, tag="ob")
            nc.scalar.mul(ob[:], c28[:], rec10[:])
            # out DMA on the ACT HWDGE queue: keeps the SP queue a pure
            # input stream (no head-of-line blocking on the epilogue)
            nc.scalar.dma_start(out_ap[b], ob[:])


_CACHE: dict = {}


def _get_nc():
    if "nc" in _CACHE:
        return _CACHE["nc"]
    nc = bacc.Bacc(
        "TRN2",
        target_bir_lowering=False,
        debug=False,
        enable_asserts=False,
        num_devices=NCORES,
    )
    f1 = nc.dram_tensor("f1", [BL, C, H, W], BF16, kind="ExternalInput").ap()
    f2 = nc.dram_tensor("f2", [BL, C, H, W], BF16, kind="ExternalInput").ap()
    wf = nc.dram_tensor("wf", [128, HW], BF16, kind="ExternalInput").ap()
    g2 = nc.dram_tensor("g2", [128, NCHUNK * K], BF16, kind="ExternalInput").ap()
    ident = nc.dram_tensor("ident", [128, 128], BF16, kind="ExternalInput").ap()
    out = nc.dram_tensor("out", [BL, K, HW28], F32, kind="ExternalOutput").ap()
    with tile.TileContext(nc) as tc:
        _build(tc, out, f1, f2, wf, g2, ident)
    nc.compile()
    _CACHE["nc"] = nc
    return nc


def kernel(feature1, feature2, knn_inds):
    f1 = np.asarray(feature1, dtype=np.float32).astype(BF16_NP)
    f2 = np.asarray(feature2, dtype=np.float32).astype(BF16_NP)
    nc = _get_nc()
    in_maps = _make_in_maps(f1, f2, knn_inds)
    res = bass_utils.run_bass_kernel_spmd(nc, in_maps, core_ids=list(range(NCORES)))
    _CACHE["last_results"] = res
    out = np.concatenate([r["out"] for r in res.results], axis=0)
    return out.reshape(B, K, S, S)
